# revision 6
# baseline (speedup 1.0000x reference)
"""FK velocity loss kernel v2 for Trainium2 (8 NeuronCores, SPMD).

Key structure (vs the v1 baseline):
  * vel_loss == pos_loss exactly => gt_prev_pose never read.
  * All compute in fp16 on-chip: ScalarE converts fp32->fp16 into a
    TRANSPOSED layout (samples contiguous innermost), which makes every DVE
    operand stride-1 in its last dim => DVE high-rate mode.
  * Instructions fused across BOTH pose tensors and BOTH chains via a
    merged tc-axis of 4 (tensor-major, chain-minor) so every DVE op needs
    at most 3 free dims (4-dim non-mergeable APs crash the device).
  * Loss reduced on device: d = z_out - z_gt, then one tensor_tensor_reduce
    (d*d, sum) per lane into a per-lane accumulator column. Host sums
    acc[P, NL] across cores in float64. No z stores.
  * One-directional engine flow (rings -> ScalarE -> DVE) - no cross-engine
    feedback, so in-order queues never ping-pong.

Layout per lane (S samples/partition):
  m32[a]: [P, S*72] f32 raw DMA (tensor a), sample-major.
  m16:    [P, 4tc, 30, S] f16, tc = tensor*2 + chain; per chain: floats
          0..26 = depth-0..2 joints verbatim (f = 9d + 3r + k),
          27..29 = t3 (c2 of the depth-3 joint).
  x16:    [P, 4tc, 3r, 3d, S] f16 cross products.
  chain:  v <- R_d v + t_d for d=2,1,0 with v init t3; all on DVE.
"""

import numpy as np

import concourse.bass as bass
import concourse.bacc as bacc
import concourse.tile as tile
from concourse import mybir

B = 262144
N_CORES = 8
PER_CORE = B // N_CORES        # 32768
P = 128
COLS = PER_CORE // P           # 256 samples per partition
F32 = mybir.dt.float32
F16 = mybir.dt.float16

DEFAULT_PLAN = (32,) * 8


def _ap(t, dims, offset=0):
    """AP over tile t with free dims [[stride,count],...] in elements."""
    base = t[:]
    return bass.AP(tensor=base.tensor, offset=base.offset + offset,
                   ap=[base.ap[0]] + [list(d) for d in dims])


def build_nc(plan=DEFAULT_PLAN, loop=None, stages="full", m16_bufs=3,
             x_bufs=1, dve_dtype=F16, pe_final=True, m32_bufs=2,
             v_bufs=1, s_bufs=1, swdge_frac=0.0, dma_ahead=3,
             conv_ahead=2):
    assert sum(plan) == COLS
    NL = len(plan)
    NACC = 2 * NL if pe_final else NL
    per_core = COLS * P

    nc = bacc.Bacc()
    src_out = nc.declare_dram_parameter("output_pose", [per_core, 72], F32,
                                        isOutput=False)
    src_gt = nc.declare_dram_parameter("gt_pose", [per_core, 72], F32,
                                       isOutput=False)
    if pe_final:
        # [I | -I] fp16 stationaries for the PE d-accumulation
        identpm_in = nc.declare_dram_parameter("identpm", [P, 2 * P],
                                               mybir.dt.float16,
                                               isOutput=False)
    acc_out = nc.declare_dram_parameter("acc_out", [P, NACC], F32,
                                        isOutput=True)

    DT = dve_dtype

    import contextlib
    with tile.TileContext(nc) as tc:
        loop_ctx = tc.For_i(0, loop, 1) if loop else contextlib.nullcontext()
        with (
            loop_ctx,
            tc.tile_pool(name="m32_pool", bufs=m32_bufs) as m32pool,
            tc.tile_pool(name="m16_pool", bufs=m16_bufs) as m16pool,
            tc.tile_pool(name="x_pool", bufs=x_bufs) as xpool,
            tc.tile_pool(name="v_pool", bufs=v_bufs) as vpool,
            tc.tile_pool(name="s_pool", bufs=s_bufs) as spool,
            tc.tile_pool(name="acc_pool", bufs=1) as accpool,
            tc.tile_pool(name="psum_pool", bufs=4, space="PSUM") as ppool,
        ):
            acc = accpool.tile([P, NACC], F32)
            if pe_final:
                identpm = accpool.tile([P, 2 * P], mybir.dt.float16)
                nc.gpsimd.dma_start(out=identpm[:], in_=identpm_in[:])

            lanes = []
            col_base = 0
            for li, S in enumerate(plan):
                lanes.append((li, S, col_base))
                col_base += S

            # ---- DMA issue: stagger K lanes ahead. The HWDGE ring has 16
            # channels and round-robins queued transfers, so issuing ALL
            # lanes up front makes every lane finish together (no early
            # data, no overlap). K in flight => lane l lands ~K transfers
            # after its issue while the ring stays fed. ----
            lane_m32 = {}

            def issue_dma(li):
                S = plan[li]
                cb = sum(plan[:li])
                row0 = cb * P
                pair = []
                for a, (src, ring) in enumerate(
                    ((src_out, nc.sync), (src_gt, nc.scalar))
                ):
                    m32 = m32pool.tile([P, S * 72], F32, tag=f"m32_{a}",
                                       name=f"m32_{a}")
                    if stages != "compute":
                        srcv = src[row0: row0 + P * S, :].rearrange(
                            "(p s) f -> p (s f)", p=P)
                        if swdge_frac > 0.0:
                            # offload the tail of each load to the otherwise
                            # idle gpsimd SWDGE ring (~170 GB/s measured)
                            h = int(S * (1.0 - swdge_frac)) * 72
                            ring.dma_start(out=m32[:, :h], in_=srcv[:, :h])
                            nc.gpsimd.dma_start(out=m32[:, h:],
                                                in_=srcv[:, h:])
                        else:
                            ring.dma_start(out=m32[:], in_=srcv)
                    pair.append(m32)
                lane_m32[li] = pair

            DMA_AHEAD = min(dma_ahead, NL)
            for li in range(DMA_AHEAD):
                issue_dma(li)

            def emit_conv(li, S):
                # ScalarE fp32 -> fp16 transposed conversion.
                # m16 [P, 4tc, 30, S]; tensor a covers tc in {2a, 2a+1}.
                # out APs keep s innermost (unit stride); in APs may have
                # any inner stride (s steps by 72 in the raw layout).
                m32s = lane_m32[li]
                m16 = m16pool.tile([P, 4, 30, S], DT, tag="m16")
                for a in range(2):
                    toff = a * 60 * S
                    nc.scalar.copy(
                        _ap(m16, [[S, 27], [30 * S, 2], [1, S]], toff),
                        _ap(m32s[a], [[1, 27], [36, 2], [72, S]], 0),
                    )
                    nc.scalar.copy(
                        _ap(m16, [[S, 3], [30 * S, 2], [1, S]],
                            toff + 27 * S),
                        _ap(m32s[a], [[3, 3], [36, 2], [72, S]], 29),
                    )
                return m16

            def emit_compute(li, S, m16):
                def mcol(d, k):
                    """m16 column k of depth d: dims (tc4, r3, S)."""
                    return _ap(m16, [[30 * S, 4], [3 * S, 3], [1, S]],
                               (9 * d + k) * S)

                # DVE: cross products x = c0 x c1, depths 0..2
                # x16 [P, 4tc, 3r, 3d, S]
                x16 = xpool.tile([P, 4, 3, 3, S], DT, tag="x")
                tmp = spool.tile([P, 4, 3, S], DT, tag="tmp")
                for r in range(3):
                    r1, r2 = (r + 1) % 3, (r + 2) % 3
                    dims_in = [[30 * S, 4], [9 * S, 3], [1, S]]  # (tc, d, S)
                    xr = _ap(x16, [[9 * S, 4], [S, 3], [1, S]], r * 3 * S)
                    nc.vector.tensor_mul(
                        xr,
                        _ap(m16, dims_in, (3 * r1 + 0) * S),
                        _ap(m16, dims_in, (3 * r2 + 1) * S))
                    nc.vector.tensor_mul(
                        tmp[:],
                        _ap(m16, dims_in, (3 * r2 + 0) * S),
                        _ap(m16, dims_in, (3 * r1 + 1) * S))
                    nc.vector.tensor_sub(xr, xr, tmp[:])
                if stages == "cross":
                    return

                # DVE chain: v <- R_d v + t_d, d = 2, 1, 0
                # v tiles [P, 4tc, 3r, S]
                def vin_t3(j):
                    return _ap(m16, [[30 * S, 4], [0, 3], [1, S]],
                               (27 + j) * S)

                def vin_v(vt, j):
                    return _ap(vt, [[3 * S, 4], [0, 3], [1, S]], j * S)

                def xd(d):
                    return _ap(x16, [[9 * S, 4], [3 * S, 3], [1, S]], d * S)

                p0 = spool.tile([P, 4, 3, S], DT, tag="p0")
                p1 = spool.tile([P, 4, 3, S], DT, tag="p1")
                va = vpool.tile([P, 4, 3, S], DT, tag="va")
                vb = vpool.tile([P, 4, 3, S], DT, tag="vb")

                def step(d, vin, vout):
                    nc.vector.tensor_mul(p0[:], xd(d), vin(0))
                    nc.vector.tensor_mul(p1[:], mcol(d, 0), vin(1))
                    nc.vector.tensor_add(p0[:], p0[:], p1[:])
                    nc.vector.tensor_mul(p1[:], mcol(d, 1), vin(2))
                    nc.vector.tensor_add(p0[:], p0[:], p1[:])
                    nc.vector.tensor_add(vout[:], p0[:], mcol(d, 2))

                step(2, vin_t3, va)
                step(1, lambda j: vin_v(va, j), vb)

                if not pe_final:
                    z = vpool.tile([P, 4, 3, S], DT, tag="z")
                    step(0, lambda j: vin_v(vb, j), z)
                    # loss: acc[:, li] = sum of (z0 - z1)^2
                    # z [P, 4tc, 3r, S]: out half tc in {0,1}, gt {2,3}
                    d16 = spool.tile([P, 2, 3, S], DT, tag="d16")
                    dsq = spool.tile([P, 2, 3, S], F32, tag="dsq")
                    zdims = [[3 * S, 2], [S, 3], [1, S]]
                    nc.vector.tensor_sub(d16[:], _ap(z, zdims, 0),
                                         _ap(z, zdims, 6 * S))
                    nc.scalar.activation(
                        dsq[:], d16[:], mybir.ActivationFunctionType.Square,
                        accum_out=acc[:, li: li + 1])
                else:
                    # step 0 products on DVE; d = z_out - z_gt accumulated
                    # directly in PSUM via [I | -I] stationaries.
                    px = spool.tile([P, 4, 3, S], DT, tag="px")
                    pc0 = spool.tile([P, 4, 3, S], DT, tag="pc0")
                    pc1 = spool.tile([P, 4, 3, S], DT, tag="pc1")
                    nc.vector.tensor_mul(px[:], xd(0), vin_v(vb, 0))
                    nc.vector.tensor_mul(pc0[:], mcol(0, 0), vin_v(vb, 1))
                    nc.vector.tensor_mul(pc1[:], mcol(0, 1), vin_v(vb, 2))
                    for c in range(2):
                        pd = ppool.tile([P, 3, S], F32, tag=f"pd{li % 4}_{c}",
                                        bufs=1, name=f"pd{li}_{c}")
                        first = True
                        for a in range(2):
                            tcix = 2 * a + c
                            stat = identpm[:, a * P:(a + 1) * P]
                            movs = [
                                _ap(px, [[S, 3], [1, S]], tcix * 3 * S),
                                _ap(pc0, [[S, 3], [1, S]], tcix * 3 * S),
                                _ap(pc1, [[S, 3], [1, S]], tcix * 3 * S),
                                _ap(m16, [[3 * S, 3], [1, S]],
                                    tcix * 30 * S + 2 * S),
                            ]
                            for mi, mov in enumerate(movs):
                                nc.tensor.matmul(
                                    pd[:], stat, mov, start=first,
                                    stop=(a == 1 and mi == 3))
                                first = False
                        pending_sq.append((li, pd, 2 * li + c))

            # ---- Phase B/C: conv skewed one lane ahead of compute;
            # squares deferred to the end so the in-order Act queue never
            # makes a later conv wait on an earlier lane's DVE+PE chain ----
            pending_sq = []
            if stages == "dma":
                for li in range(DMA_AHEAD, NL):
                    issue_dma(li)
            else:
                lane_m16 = {0: emit_conv(0, plan[0])}
                Smax = max(plan)
                dsq = spool.tile([P, 3, Smax], F32, tag="dsq")

                def drain_squares(upto):
                    # emit squares for lanes <= upto; 2 lanes late they are
                    # certainly PE-complete, so the in-order Act queue never
                    # stalls on them (and pd tags recycle mod 4 safely)
                    rest = []
                    for sli, pd, col in pending_sq:
                        if sli <= upto:
                            nc.scalar.activation(
                                dsq[:, :, 0:pd.shape[-1]], pd[:],
                                mybir.ActivationFunctionType.Square,
                                accum_out=acc[:, col: col + 1])
                        else:
                            rest.append((sli, pd, col))
                    pending_sq[:] = rest

                for k in range(1, conv_ahead):
                    if k < NL:
                        lane_m16[k] = emit_conv(k, plan[k])
                for li, S, cb in lanes:
                    if li + DMA_AHEAD < NL:
                        issue_dma(li + DMA_AHEAD)
                    if li + conv_ahead < NL:
                        lane_m16[li + conv_ahead] = emit_conv(
                            li + conv_ahead, plan[li + conv_ahead])
                    if stages != "conv":
                        emit_compute(li, S, lane_m16[li])
                    if len(pending_sq) >= 8:
                        drain_squares(NL)
                drain_squares(NL)

            if stages == "full":
                nc.gpsimd.dma_start(out=acc_out[:], in_=acc[:])
    nc.finalize()
    return nc


_NC_CACHE = {}


def _get_nc():
    if 'nc' not in _NC_CACHE:
        _NC_CACHE['nc'] = build_nc()
    return _NC_CACHE['nc']


def make_in_maps(output_pose, gt_pose, pe_final=True):
    op = np.ascontiguousarray(output_pose, dtype=np.float32)
    gt = np.ascontiguousarray(gt_pose, dtype=np.float32)
    maps = [
        {
            "output_pose": op[c * PER_CORE: (c + 1) * PER_CORE],
            "gt_pose": gt[c * PER_CORE: (c + 1) * PER_CORE],
        }
        for c in range(N_CORES)
    ]
    if pe_final:
        ident = np.eye(P, dtype=np.float16)
        identpm = np.concatenate([ident, -ident], axis=1)
        for m in maps:
            m["identpm"] = identpm
    return maps


def kernel(output_pose, gt_pose, gt_prev_pose=None, **_ignored):
    from concourse.bass_utils import run_bass_kernel_spmd
    nc = _get_nc()
    in_maps = make_in_maps(output_pose, gt_pose)
    res = run_bass_kernel_spmd(nc, in_maps, list(range(N_CORES)))
    total = 0.0
    for r in res.results:
        total += float(np.sum(r["acc_out"].astype(np.float64)))
    loss = np.float32(total / (B * 6))
    return (loss, loss)
